# revision 13
# baseline (speedup 1.0000x reference)
"""Trainium2 Bass kernel for a leaky CTRNN (nn_RNN_25451976196554).

Math (per reference):
    alpha = 1/tau; h0c = clip(h0, -1, 1); h broadcast over batch
    per step t: pre = h @ W_hh + u_t @ W_uh + b_h
                h'  = (1-alpha)*h + alpha*tanh(pre)
                y_t = softmax(h' @ W_hy + b_y)

Strategy: data-parallel over batch (8 batch rows per core, 8 cores).
Per core, three phases:
  P1: U_proj = u @ W_uh + b_h for all t (batched matmul, fp32r), -> DRAM.
  P2: sequential recurrence over T=512 steps. Batch-stationary matmuls:
      lhsT = h^T slices [128, 8] (stationary), rhs = W_hh K-tiles streamed
      as the moving operand in fp32r (1 cycle/row at N=512). The new state
      is transposed back to h^T layout with DVE 32x32 stream transposes.
  P3: y = softmax(h_hist @ W_hy + b_y), batched over all (b, t).

Layouts (per core):
  uT   [256, 4096]  : u^T, columns indexed bt = b*T + t (b-major)
  hT   [128, 512]   : h^T packed; chunk k (h dims 128k..128k+128) lives in
                      cols [32k, 32k+8) (cols 32k+8..32k+32 are zero padding
                      written by the 32x32 block transposes).
  Uc   [T, 8, 2048] : U projection, per-step slice contiguous.
  Hh   [T, 8, 2048] : h state history, per-step slice contiguous.
  y    [4096, 256]  : output rows bt = b*T + t (b-major).
"""

import numpy as np

import concourse.bass as bass
import concourse.mybir as mybir
import concourse.tile as tile
from concourse import bacc
from concourse.bass import ds
from concourse.bass_utils import run_bass_kernel_spmd
from concourse.masks import make_identity

N_IN, N_H, N_OUT = 256, 2048, 256
BATCH, T = 64, 512
NCORES = 8
BC = BATCH // NCORES          # 8 batch rows per core
BT = BC * T                   # 4096
KT = N_H // 128               # 16 K tiles over the hidden dim
KIN = N_IN // 128             # 2 K tiles over the input dim
NCH = 4                       # 512-wide psum chunks over N_H
F32 = mybir.dt.float32
F32R = mybir.dt.float32r
AF = mybir.ActivationFunctionType
STAGGERED = True
P2_DMA = True  # timing experiments only; False skips h-store/u-prefetch


def _build(fast: bool, repeat: int = 1, ncores: int = NCORES,
           rep_phases: tuple = (1, 1, 1)):
    """repeat: re-run all phases `repeat` times inside the NEFF.
    rep_phases: additional per-phase multipliers (P1, P2, P3) for
    phase-cost attribution via the wall-clock slope method.
    Output is unaffected (each rep restarts from h0)."""
    rep1, rep2, rep3 = (repeat * r for r in rep_phases)
    nc = bacc.Bacc("TRN2", target_bir_lowering=False, debug=False,
                   num_devices=ncores)

    uT = nc.declare_dram_parameter("uT", [N_IN, BT], F32, isOutput=False)
    Whh = nc.declare_dram_parameter("Whh", [N_H, N_H], F32, isOutput=False)
    Wuh = nc.declare_dram_parameter("Wuh", [N_IN, N_H], F32, isOutput=False)
    Why = nc.declare_dram_parameter("Why", [N_H, N_OUT], F32, isOutput=False)
    hT0 = nc.declare_dram_parameter("hT0", [128, KT * 32], F32, isOutput=False)
    if not fast:
        alpha_b = nc.declare_dram_parameter("alpha_b", [32, N_H], F32, isOutput=False)
        beta_b = nc.declare_dram_parameter("beta_b", [32, N_H], F32, isOutput=False)
        bh_b = nc.declare_dram_parameter("bh_b", [128, N_H], F32, isOutput=False)
        by_b = nc.declare_dram_parameter("by_b", [128, N_OUT], F32, isOutput=False)
        h0_b = nc.declare_dram_parameter("h0_b", [32, N_H], F32, isOutput=False)
    y = nc.declare_dram_parameter("y", [BT, N_OUT], F32, isOutput=True)

    Uc = nc.dram_tensor("Uc", [T, BC, N_H], F32)
    Hh = nc.dram_tensor("Hh", [T, BC, N_H], F32)
    Uc_v = Uc.ap().rearrange("t b n -> b t n")
    Hh_v = Hh.ap().rearrange("t b n -> b t n")

    with tile.TileContext(nc) as tc:
        with tc.tile_pool(name="persist", bufs=1) as persist, \
             tc.tile_pool(name="stage", bufs=2) as stage:

            # ---- P1: U projection ----
            with tc.tile_pool(name="p1", bufs=3) as p1, \
                 tc.tile_pool(name="psA", bufs=2, space="PSUM") as psA:
                wuh_r = []
                for k in range(KIN):
                    st = stage.tile([128, N_H], F32, tag="wstage")
                    nc.sync.dma_start(out=st[:, :], in_=Wuh[k * 128:(k + 1) * 128, :])
                    wr = p1.tile([128, N_H], F32R, tag=f"wuh{k}", bufs=1)
                    nc.vector.tensor_copy(wr[:, :], st[:, :])
                    wuh_r.append(wr)
                if not fast:
                    bh_sb = p1.tile([128, N_H], F32, tag="bh", bufs=1)
                    nc.sync.dma_start(out=bh_sb[:, :], in_=bh_b[:, :])

                for i in list(range(BT // 128)) * rep1:
                    b = i // 4
                    t0 = (i % 4) * 128
                    uts = []
                    for k in range(KIN):
                        st = p1.tile([128, 128], F32, tag=f"ustage{k}")
                        nc.sync.dma_start(
                            out=st[:, :],
                            in_=uT[k * 128:(k + 1) * 128, i * 128:(i + 1) * 128])
                        ur = p1.tile([128, 128], F32R, tag=f"ur{k}")
                        nc.vector.tensor_copy(ur[:, :], st[:, :])
                        uts.append(ur)
                    acc = p1.tile([128, N_H], F32, tag="uacc")
                    for ch in range(NCH):
                        ps = psA.tile([128, 512], F32, tag=f"ps{ch}")
                        for k in range(KIN):
                            nc.tensor.matmul(
                                ps[:, :], uts[k][:, :],
                                wuh_r[k][:, ch * 512:(ch + 1) * 512],
                                start=(k == 0), stop=(k == KIN - 1))
                        if fast:
                            nc.vector.tensor_copy(
                                acc[:, ch * 512:(ch + 1) * 512], ps[:, :])
                        else:
                            nc.vector.tensor_add(
                                acc[:, ch * 512:(ch + 1) * 512], ps[:, :],
                                bh_sb[:, ch * 512:(ch + 1) * 512])
                    nc.sync.dma_start(
                        out=Uc[t0:t0 + 128, b:b + 1, :], in_=acc[:, :])

            # ---- resident W_hh (rounded to fp32r for the PE) ----
            with tc.tile_pool(name="whhp", bufs=1) as whhp:
                whh_r = []
                for k in range(KT):
                    st = stage.tile([128, N_H], F32, tag="wstage")
                    nc.sync.dma_start(out=st[:, :], in_=Whh[k * 128:(k + 1) * 128, :])
                    wr = whhp.tile([128, N_H], F32R, tag=f"whh{k}")
                    nc.vector.tensor_copy(wr[:, :], st[:, :])
                    whh_r.append(wr)

                # ---- P2: recurrence ----
                # Per step: 4 psum banks of 512 cols; per bank 16 K-tile
                # matmuls, then (DVE add u) -> (ACT tanh) -> (DVE 32x32
                # transposes straight into the f32r stationary buffer).
                # The LAST bank's tail is split in two 256-col halves so the
                # next step's k=12..15 matmuls unblock sooner; everything
                # else pipelines under the next step's PE work.
                # State tiles ping-pong so the Hh store DMA and the u
                # prefetch DMAs (4 buffers, 4 steps ahead) are never on the
                # critical path.
                hT_a = persist.tile([128, KT * 32], F32R, tag="hTa")
                hT_b = persist.tile([128, KT * 32], F32R, tag="hTb")
                hT_s = persist.tile([128, KT * 32], F32, tag="hTs")
                th0 = persist.tile([32, N_H], F32, tag="th0")
                th1 = persist.tile([32, N_H], F32, tag="th1")
                nc.vector.memset(th0[0:32, :], 0.0)
                nc.vector.memset(th1[0:32, :], 0.0)
                u0 = persist.tile([BC, N_H], F32, tag="u0")
                u1 = persist.tile([BC, N_H], F32, tag="u1")
                u2 = persist.tile([BC, N_H], F32, tag="u2")
                u3 = persist.tile([BC, N_H], F32, tag="u3")
                ubufs = [u0, u1, u2, u3]
                if not fast:
                    alpha_sb = persist.tile([32, N_H], F32, tag="alpha")
                    beta_sb = persist.tile([32, N_H], F32, tag="beta")
                    hc0 = persist.tile([32, N_H], F32, tag="hc0")
                    hc1 = persist.tile([32, N_H], F32, tag="hc1")
                    nc.vector.memset(hc0[0:32, :], 0.0)
                    nc.vector.memset(hc1[0:32, :], 0.0)
                    nc.sync.dma_start(out=alpha_sb[:, :], in_=alpha_b[:, :])
                    nc.sync.dma_start(out=beta_sb[:, :], in_=beta_b[:, :])
                st = stage.tile([128, KT * 32], F32, tag="h0stage")
                nc.sync.dma_start(out=st[:, :], in_=hT0[:, :])

                with tc.tile_pool(name="psB", bufs=2, space="PSUM") as psB:

                    def step(tix, src, dst, u_tile, pf_tix, s_new, s_old,
                             th_new):
                        state = th_new if fast else s_new
                        sv = state[0:32, :].rearrange("p (i f c) -> p i f c",
                                                      f=4, c=32)

                        def tail(ps, ch, i0, ni):
                            # i0/ni are k-tile (128-col) block index/count
                            sl = slice(i0 * 128, (i0 + ni) * 128)
                            rsl = slice((i0 - 4 * ch) * 128,
                                        (i0 - 4 * ch + ni) * 128)
                            nc.vector.tensor_add(ps[:, rsl], ps[:, rsl],
                                                 u_tile[0:BC, sl])
                            nc.scalar.activation(th_new[0:BC, sl],
                                                 ps[:, rsl], AF.Tanh)
                            if not fast:
                                nc.vector.tensor_mul(th_new[0:BC, sl],
                                                     th_new[0:BC, sl],
                                                     alpha_sb[0:BC, sl])
                                nc.vector.tensor_mul(s_new[0:BC, sl],
                                                     s_old[0:BC, sl],
                                                     beta_sb[0:BC, sl])
                                nc.vector.tensor_add(s_new[0:BC, sl],
                                                     s_new[0:BC, sl],
                                                     th_new[0:BC, sl])
                            for g in range(4):
                                dv = hT_s[32 * g:32 * (g + 1), :].rearrange(
                                    "p (i c) -> p i c", c=32)
                                nc.vector.transpose(dv[:, i0:i0 + ni, :],
                                                    sv[:, i0:i0 + ni, g, :])
                            nc.vector.tensor_copy(
                                dst[:, 32 * i0:32 * (i0 + ni)],
                                hT_s[:, 32 * i0:32 * (i0 + ni)])

                        for ch in range(NCH):
                            sl = slice(ch * 512, (ch + 1) * 512)
                            ps = psB.tile([BC, 512], F32, tag=f"ps{ch}",
                                          name=f"ps{ch}")
                            for k in range(KT):
                                nc.tensor.matmul(
                                    ps[:, :], src[:, 32 * k:32 * k + BC],
                                    whh_r[k][:, sl],
                                    start=(k == 0), stop=(k == KT - 1))
                            if ch < NCH - 1:
                                tail(ps, ch, 4 * ch, 4)
                            else:
                                for j in range(4):
                                    tail(ps, ch, 4 * ch + j, 1)
                        # store state, prefetch u (both off critical path)
                        if P2_DMA:
                            nc.sync.dma_start(out=Hh_v[:, ds(tix, 1), :],
                                              in_=state[0:BC, :])
                            if pf_tix is not None:
                                nc.sync.dma_start(out=u_tile[0:BC, :],
                                                  in_=Uc_v[:, ds(pf_tix, 1), :])

                    def estep(t, pf_tix):
                        src, dst = (hT_a, hT_b) if t % 2 == 0 else (hT_b, hT_a)
                        s_new, s_old = ((hc0, hc1) if t % 2 == 0 else
                                        (hc1, hc0)) if not fast else (None, None)
                        th_new = th0 if t % 2 == 0 else th1
                        step(t, src, dst, ubufs[t % 4], pf_tix, s_new, s_old,
                             th_new)

                    for _rep in range(rep2):
                        nc.vector.tensor_copy(hT_a[:, :], st[:, :])
                        if not fast:
                            nc.sync.dma_start(out=hc1[:, :], in_=h0_b[:, :])
                        for j in range(4):
                            nc.sync.dma_start(out=ubufs[j][0:BC, :],
                                              in_=Uc_v[:, ds(j, 1), :])
                        with tc.For_i(0, (T - 16) // 4, 1,
                                      staggered_reset=STAGGERED) as iv:
                            t0 = iv * 4
                            for j in range(4):
                                estep(t0 + j, t0 + j + 4)
                        for t in range(T - 16, T):
                            estep(t, t + 4 if t + 4 < T else None)

            # ---- P3: output projection + softmax ----
            with tc.tile_pool(name="p3", bufs=3) as p3, \
                 tc.tile_pool(name="psC", bufs=2, space="PSUM") as psC:
                why_r = []
                for k in range(KT):
                    st = stage.tile([128, N_OUT], F32, tag="whystage")
                    nc.sync.dma_start(out=st[:, :],
                                      in_=Why[k * 128:(k + 1) * 128, :])
                    wr = p3.tile([128, N_OUT], F32R, tag=f"why{k}", bufs=1)
                    nc.vector.tensor_copy(wr[:, :], st[:, :])
                    why_r.append(wr)
                ident = p3.tile([128, 128], F32, tag="ident", bufs=1)
                make_identity(nc, ident[:, :])
                if not fast:
                    by_sb = p3.tile([128, N_OUT], F32, tag="by", bufs=1)
                    nc.sync.dma_start(out=by_sb[:, :], in_=by_b[:, :])

                for i in list(range(BT // 128)) * rep3:
                    b = i // 4
                    t0 = (i % 4) * 128
                    htile = p3.tile([128, N_H], F32, tag="h3")
                    nc.sync.dma_start(out=htile[:, :],
                                      in_=Hh[t0:t0 + 128, b:b + 1, :])
                    hT3 = p3.tile([128, N_H], F32R, tag="hT3", bufs=2)
                    psy = psC.tile([128, N_OUT], F32, tag="psy")
                    for k in range(KT):
                        pst = psC.tile([128, 128], F32, tag=f"pst{k % 2}")
                        nc.tensor.transpose(
                            pst[:, :], htile[:, k * 128:(k + 1) * 128],
                            ident[:, :])
                        nc.vector.tensor_copy(
                            hT3[:, k * 128:(k + 1) * 128], pst[:, :])
                    for k in range(KT):
                        nc.tensor.matmul(
                            psy[:, :], hT3[:, k * 128:(k + 1) * 128],
                            why_r[k][:, :], start=(k == 0), stop=(k == KT - 1))
                    logits = p3.tile([128, N_OUT], F32, tag="logits")
                    if fast:
                        nc.vector.tensor_copy(logits[:, :], psy[:, :])
                    else:
                        nc.vector.tensor_add(logits[:, :], psy[:, :],
                                             by_sb[:, :])
                    nm = p3.tile([128, 1], F32, tag="nm")
                    nc.vector.tensor_reduce(nm[:, :], logits[:, :],
                                            axis=mybir.AxisListType.X,
                                            op=mybir.AluOpType.max)
                    nc.vector.tensor_scalar_mul(nm[:, :], nm[:, :], -1.0)
                    e = p3.tile([128, N_OUT], F32, tag="e")
                    s = p3.tile([128, 1], F32, tag="s")
                    nc.scalar.activation(e[:, :], logits[:, :], AF.Exp,
                                         bias=nm[:, :], scale=1.0,
                                         accum_out=s[:, :])
                    r = p3.tile([128, 1], F32, tag="r")
                    nc.vector.reciprocal(r[:, :], s[:, :])
                    yt = p3.tile([128, N_OUT], F32, tag="yt")
                    nc.vector.tensor_scalar_mul(yt[:, :], e[:, :], r[:, :])
                    nc.sync.dma_start(out=y[i * 128:(i + 1) * 128, :],
                                      in_=yt[:, :])

    nc.compile()
    return nc


_NC_CACHE = {}


def _get_nc(fast: bool):
    if fast not in _NC_CACHE:
        _NC_CACHE[fast] = _build(fast)
    return _NC_CACHE[fast]


def make_in_maps(inputs):
    """Build per-core input maps for the `fast` variant (setup_inputs data)."""
    u = np.ascontiguousarray(np.asarray(inputs["u"], dtype=np.float32))
    common, _ = _prep_common(
        inputs["W_uh"], inputs["W_hh"], inputs["W_hy"], inputs["b_h"],
        inputs["b_y"], inputs["h0"], inputs["tau"])
    in_maps = []
    for c in range(NCORES):
        uc = u[c * BC:(c + 1) * BC]
        uTc = np.ascontiguousarray(uc.reshape(BC * T, N_IN).T)
        in_maps.append({"uT": uTc, **common})
    return in_maps


def _prep_common(W_uh, W_hh, W_hy, b_h, b_y, h0, tau):
    W_uh = np.ascontiguousarray(np.asarray(W_uh, dtype=np.float32))
    W_hh = np.ascontiguousarray(np.asarray(W_hh, dtype=np.float32))
    W_hy = np.ascontiguousarray(np.asarray(W_hy, dtype=np.float32))
    b_h = np.asarray(b_h, dtype=np.float32)
    b_y = np.asarray(b_y, dtype=np.float32)
    h0 = np.asarray(h0, dtype=np.float32)
    tau = np.asarray(tau, dtype=np.float32)

    alpha = 1.0 / tau
    fast = bool(np.all(alpha == 1.0) and np.all(b_h == 0.0) and np.all(b_y == 0.0))

    h0c = np.clip(h0, -1.0, 1.0)
    hT0 = np.zeros((128, KT * 32), np.float32)
    for k in range(KT):
        hT0[:, 32 * k:32 * k + BC] = h0c[128 * k:128 * (k + 1)][:, None]

    common = {"Whh": W_hh, "Wuh": W_uh, "Why": W_hy, "hT0": hT0}
    if not fast:
        alpha_b = np.zeros((32, N_H), np.float32)
        alpha_b[:BC] = alpha[None, :]
        beta_b = np.zeros((32, N_H), np.float32)
        beta_b[:BC] = (1.0 - alpha)[None, :]
        h0_b = np.zeros((32, N_H), np.float32)
        h0_b[:BC] = h0c[None, :]
        common.update(
            alpha_b=alpha_b, beta_b=beta_b, h0_b=h0_b,
            bh_b=np.ascontiguousarray(np.broadcast_to(b_h[None, :], (128, N_H))),
            by_b=np.ascontiguousarray(np.broadcast_to(b_y[None, :], (128, N_OUT))),
        )
    return common, fast


def kernel(u, W_uh, W_hh, W_hy, b_h, b_y, h0, tau):
    u = np.ascontiguousarray(np.asarray(u, dtype=np.float32))
    common, fast = _prep_common(W_uh, W_hh, W_hy, b_h, b_y, h0, tau)
    nc = _get_nc(fast)

    in_maps = []
    for c in range(NCORES):
        uc = u[c * BC:(c + 1) * BC]                      # [BC, T, N_IN]
        uTc = np.ascontiguousarray(uc.reshape(BC * T, N_IN).T)
        in_maps.append({"uT": uTc, **common})

    res = run_bass_kernel_spmd(nc, in_maps, core_ids=list(range(NCORES)))
    ys = [res.results[c]["y"].reshape(BC, T, N_OUT) for c in range(NCORES)]
    return np.concatenate(ys, axis=0)



# revision 14
# speedup vs baseline: 1.1813x; 1.1813x over previous
"""Trainium2 Bass kernel for a leaky CTRNN (nn_RNN_25451976196554).

Math (per reference):
    alpha = 1/tau; h0c = clip(h0, -1, 1); h broadcast over batch
    per step t: pre = h @ W_hh + u_t @ W_uh + b_h
                h'  = (1-alpha)*h + alpha*tanh(pre)
                y_t = softmax(h' @ W_hy + b_y)

Strategy: data-parallel over batch (8 batch rows per core, 8 cores).
Per core, three phases:
  P1: U_proj = u @ W_uh + b_h for all t (batched matmul, fp32r), -> DRAM.
  P2: sequential recurrence over T=512 steps. Batch-stationary matmuls:
      lhsT = h^T slices [128, 8] (stationary), rhs = W_hh K-tiles streamed
      as the moving operand in fp32r (1 cycle/row at N=512). The new state
      is transposed back to h^T layout with DVE 32x32 stream transposes.
  P3: y = softmax(h_hist @ W_hy + b_y), batched over all (b, t).

Layouts (per core):
  uT   [256, 4096]  : u^T, columns indexed bt = b*T + t (b-major)
  hT   [128, 512]   : h^T packed; chunk k (h dims 128k..128k+128) lives in
                      cols [32k, 32k+8) (cols 32k+8..32k+32 are zero padding
                      written by the 32x32 block transposes).
  Uc   [T, 8, 2048] : U projection, per-step slice contiguous.
  Hh   [T, 8, 2048] : h state history, per-step slice contiguous.
  y    [4096, 256]  : output rows bt = b*T + t (b-major).
"""

import numpy as np

import concourse.bass as bass
import concourse.mybir as mybir
import concourse.tile as tile
from concourse import bacc
from concourse.bass import ds
from concourse.bass_utils import run_bass_kernel_spmd
from concourse.masks import make_identity

N_IN, N_H, N_OUT = 256, 2048, 256
BATCH, T = 64, 512
NCORES = 8
BC = BATCH // NCORES          # 8 batch rows per core
BT = BC * T                   # 4096
KT = N_H // 128               # 16 K tiles over the hidden dim
KIN = N_IN // 128             # 2 K tiles over the input dim
NCH = 4                       # 512-wide psum chunks over N_H
F32 = mybir.dt.float32
F32R = mybir.dt.float32r
AF = mybir.ActivationFunctionType
STAGGERED = True
P2_DMA = True  # timing experiments only; False skips h-store/u-prefetch


def _build(fast: bool, repeat: int = 1, ncores: int = NCORES,
           rep_phases: tuple = (1, 1, 1)):
    """repeat: re-run all phases `repeat` times inside the NEFF.
    rep_phases: additional per-phase multipliers (P1, P2, P3) for
    phase-cost attribution via the wall-clock slope method.
    Output is unaffected (each rep restarts from h0)."""
    rep1, rep2, rep3 = (repeat * r for r in rep_phases)
    nc = bacc.Bacc("TRN2", target_bir_lowering=False, debug=False,
                   num_devices=ncores)

    uT = nc.declare_dram_parameter("uT", [N_IN, BT], F32, isOutput=False)
    Whh = nc.declare_dram_parameter("Whh", [N_H, N_H], F32, isOutput=False)
    Wuh = nc.declare_dram_parameter("Wuh", [N_IN, N_H], F32, isOutput=False)
    Why = nc.declare_dram_parameter("Why", [N_H, N_OUT], F32, isOutput=False)
    hT0 = nc.declare_dram_parameter("hT0", [128, KT * 32], F32, isOutput=False)
    if not fast:
        alpha_b = nc.declare_dram_parameter("alpha_b", [32, N_H], F32, isOutput=False)
        beta_b = nc.declare_dram_parameter("beta_b", [32, N_H], F32, isOutput=False)
        bh_b = nc.declare_dram_parameter("bh_b", [128, N_H], F32, isOutput=False)
        by_b = nc.declare_dram_parameter("by_b", [128, N_OUT], F32, isOutput=False)
        h0_b = nc.declare_dram_parameter("h0_b", [32, N_H], F32, isOutput=False)
    y = nc.declare_dram_parameter("y", [BT, N_OUT], F32, isOutput=True)

    Uc = nc.dram_tensor("Uc", [T, BC, N_H], F32)
    Hh = nc.dram_tensor("Hh", [T, BC, N_H], F32)
    Uc_v = Uc.ap().rearrange("t b n -> b t n")
    Hh_v = Hh.ap().rearrange("t b n -> b t n")

    with tile.TileContext(nc) as tc:
        with tc.tile_pool(name="persist", bufs=1) as persist, \
             tc.tile_pool(name="stage", bufs=2) as stage:

            # ---- P1: U projection ----
            with tc.tile_pool(name="p1", bufs=3) as p1, \
                 tc.tile_pool(name="psA", bufs=2, space="PSUM") as psA:
                wuh_r = []
                for k in range(KIN):
                    st = stage.tile([128, N_H], F32, tag="wstage")
                    nc.sync.dma_start(out=st[:, :], in_=Wuh[k * 128:(k + 1) * 128, :])
                    wr = p1.tile([128, N_H], F32R, tag=f"wuh{k}", bufs=1)
                    nc.vector.tensor_copy(wr[:, :], st[:, :])
                    wuh_r.append(wr)
                if not fast:
                    bh_sb = p1.tile([128, N_H], F32, tag="bh", bufs=1)
                    nc.sync.dma_start(out=bh_sb[:, :], in_=bh_b[:, :])

                for i in list(range(BT // 128)) * rep1:
                    b = i // 4
                    t0 = (i % 4) * 128
                    uts = []
                    for k in range(KIN):
                        st = p1.tile([128, 128], F32, tag=f"ustage{k}")
                        nc.sync.dma_start(
                            out=st[:, :],
                            in_=uT[k * 128:(k + 1) * 128, i * 128:(i + 1) * 128])
                        ur = p1.tile([128, 128], F32R, tag=f"ur{k}")
                        nc.vector.tensor_copy(ur[:, :], st[:, :])
                        uts.append(ur)
                    acc = p1.tile([128, N_H], F32, tag="uacc")
                    for ch in range(NCH):
                        ps = psA.tile([128, 512], F32, tag=f"ps{ch}")
                        for k in range(KIN):
                            nc.tensor.matmul(
                                ps[:, :], uts[k][:, :],
                                wuh_r[k][:, ch * 512:(ch + 1) * 512],
                                start=(k == 0), stop=(k == KIN - 1))
                        if fast:
                            nc.vector.tensor_copy(
                                acc[:, ch * 512:(ch + 1) * 512], ps[:, :])
                        else:
                            nc.vector.tensor_add(
                                acc[:, ch * 512:(ch + 1) * 512], ps[:, :],
                                bh_sb[:, ch * 512:(ch + 1) * 512])
                    nc.sync.dma_start(
                        out=Uc[t0:t0 + 128, b:b + 1, :], in_=acc[:, :])

            # ---- resident W_hh (rounded to fp32r for the PE) ----
            with tc.tile_pool(name="whhp", bufs=1) as whhp:
                whh_r = []
                for k in range(KT):
                    st = stage.tile([128, N_H], F32, tag="wstage")
                    nc.sync.dma_start(out=st[:, :], in_=Whh[k * 128:(k + 1) * 128, :])
                    wr = whhp.tile([128, N_H], F32R, tag=f"whh{k}")
                    nc.vector.tensor_copy(wr[:, :], st[:, :])
                    whh_r.append(wr)

                # ---- P2: recurrence ----
                # Per step: 4 psum banks of 512 cols; per bank 16 K-tile
                # matmuls, then (DVE add u) -> (ACT tanh) -> (DVE 32x32
                # transposes straight into the f32r stationary buffer).
                # The LAST bank's tail is split in two 256-col halves so the
                # next step's k=12..15 matmuls unblock sooner; everything
                # else pipelines under the next step's PE work.
                # State tiles ping-pong so the Hh store DMA and the u
                # prefetch DMAs (4 buffers, 4 steps ahead) are never on the
                # critical path.
                hT_a = persist.tile([128, KT * 32], F32R, tag="hTa")
                hT_b = persist.tile([128, KT * 32], F32R, tag="hTb")
                hT_s = persist.tile([128, KT * 32], F32, tag="hTs")
                th0 = persist.tile([32, N_H], F32, tag="th0")
                th1 = persist.tile([32, N_H], F32, tag="th1")
                nc.vector.memset(th0[0:32, :], 0.0)
                nc.vector.memset(th1[0:32, :], 0.0)
                u0 = persist.tile([BC, N_H], F32, tag="u0")
                u1 = persist.tile([BC, N_H], F32, tag="u1")
                u2 = persist.tile([BC, N_H], F32, tag="u2")
                u3 = persist.tile([BC, N_H], F32, tag="u3")
                ubufs = [u0, u1, u2, u3]
                if not fast:
                    alpha_sb = persist.tile([32, N_H], F32, tag="alpha")
                    beta_sb = persist.tile([32, N_H], F32, tag="beta")
                    hc0 = persist.tile([32, N_H], F32, tag="hc0")
                    hc1 = persist.tile([32, N_H], F32, tag="hc1")
                    nc.vector.memset(hc0[0:32, :], 0.0)
                    nc.vector.memset(hc1[0:32, :], 0.0)
                    nc.sync.dma_start(out=alpha_sb[:, :], in_=alpha_b[:, :])
                    nc.sync.dma_start(out=beta_sb[:, :], in_=beta_b[:, :])
                st = stage.tile([128, KT * 32], F32, tag="h0stage")
                nc.sync.dma_start(out=st[:, :], in_=hT0[:, :])

                with tc.tile_pool(name="psB", bufs=2, space="PSUM") as psB:

                    def step(tix, src, dst, u_tile, pf_tix, s_new, s_old,
                             th_new):
                        state = th_new if fast else s_new
                        sv = state[0:32, :].rearrange("p (i f c) -> p i f c",
                                                      f=4, c=32)

                        def tail(ps, ch, i0, ni):
                            # i0/ni are k-tile (128-col) block index/count
                            sl = slice(i0 * 128, (i0 + ni) * 128)
                            rsl = slice((i0 - 4 * ch) * 128,
                                        (i0 - 4 * ch + ni) * 128)
                            nc.vector.tensor_add(ps[:, rsl], ps[:, rsl],
                                                 u_tile[0:BC, sl])
                            nc.scalar.activation(th_new[0:BC, sl],
                                                 ps[:, rsl], AF.Tanh)
                            if not fast:
                                nc.vector.tensor_mul(th_new[0:BC, sl],
                                                     th_new[0:BC, sl],
                                                     alpha_sb[0:BC, sl])
                                nc.vector.tensor_mul(s_new[0:BC, sl],
                                                     s_old[0:BC, sl],
                                                     beta_sb[0:BC, sl])
                                nc.vector.tensor_add(s_new[0:BC, sl],
                                                     s_new[0:BC, sl],
                                                     th_new[0:BC, sl])
                            for g in range(4):
                                dv = hT_s[32 * g:32 * (g + 1), :].rearrange(
                                    "p (i c) -> p i c", c=32)
                                nc.vector.transpose(dv[:, i0:i0 + ni, :],
                                                    sv[:, i0:i0 + ni, g, :])
                            nc.vector.tensor_copy(
                                dst[:, 32 * i0:32 * (i0 + ni)],
                                hT_s[:, 32 * i0:32 * (i0 + ni)])

                        for ch in range(NCH):
                            sl = slice(ch * 512, (ch + 1) * 512)
                            ps = psB.tile([BC, 512], F32, tag=f"ps{ch}",
                                          name=f"ps{ch}")
                            for k in range(KT):
                                nc.tensor.matmul(
                                    ps[:, :], src[:, 32 * k:32 * k + BC],
                                    whh_r[k][:, sl],
                                    start=(k == 0), stop=(k == KT - 1))
                            if ch < NCH - 1:
                                tail(ps, ch, 4 * ch, 4)
                            else:
                                tail(ps, ch, 4 * ch, 2)
                                tail(ps, ch, 4 * ch + 2, 2)
                        # store state, prefetch u (both off critical path)
                        if P2_DMA:
                            nc.sync.dma_start(out=Hh_v[:, ds(tix, 1), :],
                                              in_=state[0:BC, :])
                            if pf_tix is not None:
                                nc.sync.dma_start(out=u_tile[0:BC, :],
                                                  in_=Uc_v[:, ds(pf_tix, 1), :])

                    def estep(t, pf_tix):
                        src, dst = (hT_a, hT_b) if t % 2 == 0 else (hT_b, hT_a)
                        s_new, s_old = ((hc0, hc1) if t % 2 == 0 else
                                        (hc1, hc0)) if not fast else (None, None)
                        th_new = th0 if t % 2 == 0 else th1
                        step(t, src, dst, ubufs[t % 4], pf_tix, s_new, s_old,
                             th_new)

                    for _rep in range(rep2):
                        nc.vector.tensor_copy(hT_a[:, :], st[:, :])
                        if not fast:
                            nc.sync.dma_start(out=hc1[:, :], in_=h0_b[:, :])
                        for j in range(4):
                            nc.sync.dma_start(out=ubufs[j][0:BC, :],
                                              in_=Uc_v[:, ds(j, 1), :])
                        with tc.For_i(0, (T - 16) // 4, 1,
                                      staggered_reset=STAGGERED) as iv:
                            t0 = iv * 4
                            for j in range(4):
                                estep(t0 + j, t0 + j + 4)
                        for t in range(T - 16, T):
                            estep(t, t + 4 if t + 4 < T else None)

            # ---- P3: output projection + softmax ----
            with tc.tile_pool(name="p3", bufs=3) as p3, \
                 tc.tile_pool(name="psC", bufs=2, space="PSUM") as psC:
                why_r = []
                for k in range(KT):
                    st = stage.tile([128, N_OUT], F32, tag="whystage")
                    nc.sync.dma_start(out=st[:, :],
                                      in_=Why[k * 128:(k + 1) * 128, :])
                    wr = p3.tile([128, N_OUT], F32R, tag=f"why{k}", bufs=1)
                    nc.vector.tensor_copy(wr[:, :], st[:, :])
                    why_r.append(wr)
                ident = p3.tile([128, 128], F32, tag="ident", bufs=1)
                make_identity(nc, ident[:, :])
                if not fast:
                    by_sb = p3.tile([128, N_OUT], F32, tag="by", bufs=1)
                    nc.sync.dma_start(out=by_sb[:, :], in_=by_b[:, :])

                for i in list(range(BT // 128)) * rep3:
                    b = i // 4
                    t0 = (i % 4) * 128
                    htile = p3.tile([128, N_H], F32, tag="h3")
                    nc.sync.dma_start(out=htile[:, :],
                                      in_=Hh[t0:t0 + 128, b:b + 1, :])
                    hT3 = p3.tile([128, N_H], F32R, tag="hT3", bufs=2)
                    psy = psC.tile([128, N_OUT], F32, tag="psy")
                    for k in range(KT):
                        pst = psC.tile([128, 128], F32, tag=f"pst{k % 2}")
                        nc.tensor.transpose(
                            pst[:, :], htile[:, k * 128:(k + 1) * 128],
                            ident[:, :])
                        nc.vector.tensor_copy(
                            hT3[:, k * 128:(k + 1) * 128], pst[:, :])
                    for k in range(KT):
                        nc.tensor.matmul(
                            psy[:, :], hT3[:, k * 128:(k + 1) * 128],
                            why_r[k][:, :], start=(k == 0), stop=(k == KT - 1))
                    logits = p3.tile([128, N_OUT], F32, tag="logits")
                    if fast:
                        nc.vector.tensor_copy(logits[:, :], psy[:, :])
                    else:
                        nc.vector.tensor_add(logits[:, :], psy[:, :],
                                             by_sb[:, :])
                    nm = p3.tile([128, 1], F32, tag="nm")
                    nc.vector.tensor_reduce(nm[:, :], logits[:, :],
                                            axis=mybir.AxisListType.X,
                                            op=mybir.AluOpType.max)
                    nc.vector.tensor_scalar_mul(nm[:, :], nm[:, :], -1.0)
                    e = p3.tile([128, N_OUT], F32, tag="e")
                    s = p3.tile([128, 1], F32, tag="s")
                    nc.scalar.activation(e[:, :], logits[:, :], AF.Exp,
                                         bias=nm[:, :], scale=1.0,
                                         accum_out=s[:, :])
                    r = p3.tile([128, 1], F32, tag="r")
                    nc.vector.reciprocal(r[:, :], s[:, :])
                    yt = p3.tile([128, N_OUT], F32, tag="yt")
                    nc.vector.tensor_scalar_mul(yt[:, :], e[:, :], r[:, :])
                    nc.sync.dma_start(out=y[i * 128:(i + 1) * 128, :],
                                      in_=yt[:, :])

    nc.compile()
    return nc


_NC_CACHE = {}


def _get_nc(fast: bool):
    if fast not in _NC_CACHE:
        _NC_CACHE[fast] = _build(fast)
    return _NC_CACHE[fast]


def make_in_maps(inputs):
    """Build per-core input maps for the `fast` variant (setup_inputs data)."""
    u = np.ascontiguousarray(np.asarray(inputs["u"], dtype=np.float32))
    common, _ = _prep_common(
        inputs["W_uh"], inputs["W_hh"], inputs["W_hy"], inputs["b_h"],
        inputs["b_y"], inputs["h0"], inputs["tau"])
    in_maps = []
    for c in range(NCORES):
        uc = u[c * BC:(c + 1) * BC]
        uTc = np.ascontiguousarray(uc.reshape(BC * T, N_IN).T)
        in_maps.append({"uT": uTc, **common})
    return in_maps


def _prep_common(W_uh, W_hh, W_hy, b_h, b_y, h0, tau):
    W_uh = np.ascontiguousarray(np.asarray(W_uh, dtype=np.float32))
    W_hh = np.ascontiguousarray(np.asarray(W_hh, dtype=np.float32))
    W_hy = np.ascontiguousarray(np.asarray(W_hy, dtype=np.float32))
    b_h = np.asarray(b_h, dtype=np.float32)
    b_y = np.asarray(b_y, dtype=np.float32)
    h0 = np.asarray(h0, dtype=np.float32)
    tau = np.asarray(tau, dtype=np.float32)

    alpha = 1.0 / tau
    fast = bool(np.all(alpha == 1.0) and np.all(b_h == 0.0) and np.all(b_y == 0.0))

    h0c = np.clip(h0, -1.0, 1.0)
    hT0 = np.zeros((128, KT * 32), np.float32)
    for k in range(KT):
        hT0[:, 32 * k:32 * k + BC] = h0c[128 * k:128 * (k + 1)][:, None]

    common = {"Whh": W_hh, "Wuh": W_uh, "Why": W_hy, "hT0": hT0}
    if not fast:
        alpha_b = np.zeros((32, N_H), np.float32)
        alpha_b[:BC] = alpha[None, :]
        beta_b = np.zeros((32, N_H), np.float32)
        beta_b[:BC] = (1.0 - alpha)[None, :]
        h0_b = np.zeros((32, N_H), np.float32)
        h0_b[:BC] = h0c[None, :]
        common.update(
            alpha_b=alpha_b, beta_b=beta_b, h0_b=h0_b,
            bh_b=np.ascontiguousarray(np.broadcast_to(b_h[None, :], (128, N_H))),
            by_b=np.ascontiguousarray(np.broadcast_to(b_y[None, :], (128, N_OUT))),
        )
    return common, fast


def kernel(u, W_uh, W_hh, W_hy, b_h, b_y, h0, tau):
    u = np.ascontiguousarray(np.asarray(u, dtype=np.float32))
    common, fast = _prep_common(W_uh, W_hh, W_hy, b_h, b_y, h0, tau)
    nc = _get_nc(fast)

    in_maps = []
    for c in range(NCORES):
        uc = u[c * BC:(c + 1) * BC]                      # [BC, T, N_IN]
        uTc = np.ascontiguousarray(uc.reshape(BC * T, N_IN).T)
        in_maps.append({"uT": uTc, **common})

    res = run_bass_kernel_spmd(nc, in_maps, core_ids=list(range(NCORES)))
    ys = [res.results[c]["y"].reshape(BC, T, N_OUT) for c in range(NCORES)]
    return np.concatenate(ys, axis=0)



# revision 16
# speedup vs baseline: 1.3954x; 1.1812x over previous
"""Trainium2 Bass kernel for a leaky CTRNN (nn_RNN_25451976196554).

Math (per reference):
    alpha = 1/tau; h0c = clip(h0, -1, 1); h broadcast over batch
    per step t: pre = h @ W_hh + u_t @ W_uh + b_h
                h'  = (1-alpha)*h + alpha*tanh(pre)
                y_t = softmax(h' @ W_hy + b_y)

Strategy: data-parallel over batch (8 batch rows per core, 8 cores).
Tensor-parallelism (per the sharding hint) was measured and rejected: in
this environment a single 8-core AllGather costs ~270us and a critical-
path DMA round ~30us, so any per-step cross-core exchange dwarfs the
1.7us/step it would save on the PE.  Per core, three phases:
  P1: U_proj = u @ W_uh + b_h for all t (batched matmul, fp32r), -> DRAM.
  P2: sequential recurrence over T=512 steps. Batch-stationary matmuls:
      lhsT = h^T slices [128, 8] (stationary), rhs = W_hh K-tiles streamed
      as the moving operand in fp32r (1 cycle/row at N=512; measured
      ~15.3us/step pure-PE floor, cost ~ N_moving only). Per 512-wide
      psum bank: 16 matmuls, then DVE add-u (in place on PSUM), ACT tanh,
      DVE 32x32 transposes into an f32 staging buffer, and a rounding
      copy into the f32r stationary for the next step.  The last bank's
      tail is split in two 256-col halves so the next step's k=12..15
      matmuls unblock early; state/psum tiles ping-pong and u is
      prefetched 4 steps ahead so no DMA sits on the critical path.
      (fp8 DoubleRow would halve PE streaming time but fails the 2e-2
      gate: simulated rel err 2.6e-2. k-major matmul emission that
      interleaves PSUM accumulation groups is 2.4x slower - banks want
      16 consecutive matmuls.)
  P3: y = softmax(h_hist @ W_hy + b_y), batched over all (b, t).

Layouts (per core):
  uT   [256, 4096]  : u^T, columns indexed bt = b*T + t (b-major)
  hT   [128, 512]   : h^T packed; chunk k (h dims 128k..128k+128) lives in
                      cols [32k, 32k+8) (cols 32k+8..32k+32 are zero padding
                      written by the 32x32 block transposes).
  Uc   [T, 8, 2048] : U projection, per-step slice contiguous.
  Hh   [T, 8, 2048] : h state history, per-step slice contiguous.
  y    [4096, 256]  : output rows bt = b*T + t (b-major).
"""

import numpy as np

import concourse.bass as bass
import concourse.mybir as mybir
import concourse.tile as tile
from concourse import bacc
from concourse.bass import ds
from concourse.bass_utils import run_bass_kernel_spmd
from concourse.masks import make_identity

N_IN, N_H, N_OUT = 256, 2048, 256
BATCH, T = 64, 512
NCORES = 8
BC = BATCH // NCORES          # 8 batch rows per core
BT = BC * T                   # 4096
KT = N_H // 128               # 16 K tiles over the hidden dim
KIN = N_IN // 128             # 2 K tiles over the input dim
NCH = 4                       # 512-wide psum chunks over N_H
F32 = mybir.dt.float32
F32R = mybir.dt.float32r
AF = mybir.ActivationFunctionType
STAGGERED = True
P2_DMA = True  # timing experiments only; False skips h-store/u-prefetch


def _build(fast: bool, repeat: int = 1, ncores: int = NCORES,
           rep_phases: tuple = (1, 1, 1)):
    """repeat: re-run all phases `repeat` times inside the NEFF.
    rep_phases: additional per-phase multipliers (P1, P2, P3) for
    phase-cost attribution via the wall-clock slope method.
    Output is unaffected (each rep restarts from h0)."""
    rep1, rep2, rep3 = (repeat * r for r in rep_phases)
    nc = bacc.Bacc("TRN2", target_bir_lowering=False, debug=False,
                   num_devices=ncores)

    uT = nc.declare_dram_parameter("uT", [N_IN, BT], F32, isOutput=False)
    Whh = nc.declare_dram_parameter("Whh", [N_H, N_H], F32, isOutput=False)
    Wuh = nc.declare_dram_parameter("Wuh", [N_IN, N_H], F32, isOutput=False)
    Why = nc.declare_dram_parameter("Why", [N_H, N_OUT], F32, isOutput=False)
    hT0 = nc.declare_dram_parameter("hT0", [128, KT * 32], F32, isOutput=False)
    if not fast:
        alpha_b = nc.declare_dram_parameter("alpha_b", [32, N_H], F32, isOutput=False)
        beta_b = nc.declare_dram_parameter("beta_b", [32, N_H], F32, isOutput=False)
        bh_b = nc.declare_dram_parameter("bh_b", [128, N_H], F32, isOutput=False)
        by_b = nc.declare_dram_parameter("by_b", [128, N_OUT], F32, isOutput=False)
        h0_b = nc.declare_dram_parameter("h0_b", [32, N_H], F32, isOutput=False)
    y = nc.declare_dram_parameter("y", [BT, N_OUT], F32, isOutput=True)

    Uc = nc.dram_tensor("Uc", [T, BC, N_H], F32)
    Hh = nc.dram_tensor("Hh", [T, BC, N_H], F32)
    Uc_v = Uc.ap().rearrange("t b n -> b t n")
    Hh_v = Hh.ap().rearrange("t b n -> b t n")

    with tile.TileContext(nc) as tc:
        with tc.tile_pool(name="persist", bufs=1) as persist, \
             tc.tile_pool(name="stage", bufs=2) as stage:

            # ---- P1: U projection ----
            with tc.tile_pool(name="p1", bufs=3) as p1, \
                 tc.tile_pool(name="psA", bufs=2, space="PSUM") as psA:
                wuh_r = []
                for k in range(KIN):
                    st = stage.tile([128, N_H], F32, tag="wstage")
                    nc.sync.dma_start(out=st[:, :], in_=Wuh[k * 128:(k + 1) * 128, :])
                    wr = p1.tile([128, N_H], F32R, tag=f"wuh{k}", bufs=1)
                    nc.vector.tensor_copy(wr[:, :], st[:, :])
                    wuh_r.append(wr)
                if not fast:
                    bh_sb = p1.tile([128, N_H], F32, tag="bh", bufs=1)
                    nc.sync.dma_start(out=bh_sb[:, :], in_=bh_b[:, :])

                for i in list(range(BT // 128)) * rep1:
                    b = i // 4
                    t0 = (i % 4) * 128
                    uts = []
                    for k in range(KIN):
                        st = p1.tile([128, 128], F32, tag=f"ustage{k}")
                        nc.sync.dma_start(
                            out=st[:, :],
                            in_=uT[k * 128:(k + 1) * 128, i * 128:(i + 1) * 128])
                        ur = p1.tile([128, 128], F32R, tag=f"ur{k}")
                        nc.vector.tensor_copy(ur[:, :], st[:, :])
                        uts.append(ur)
                    acc = p1.tile([128, N_H], F32, tag="uacc")
                    for ch in range(NCH):
                        ps = psA.tile([128, 512], F32, tag=f"ps{ch}")
                        for k in range(KIN):
                            nc.tensor.matmul(
                                ps[:, :], uts[k][:, :],
                                wuh_r[k][:, ch * 512:(ch + 1) * 512],
                                start=(k == 0), stop=(k == KIN - 1))
                        if fast:
                            nc.vector.tensor_copy(
                                acc[:, ch * 512:(ch + 1) * 512], ps[:, :])
                        else:
                            nc.vector.tensor_add(
                                acc[:, ch * 512:(ch + 1) * 512], ps[:, :],
                                bh_sb[:, ch * 512:(ch + 1) * 512])
                    nc.sync.dma_start(
                        out=Uc[t0:t0 + 128, b:b + 1, :], in_=acc[:, :])

            # ---- resident W_hh (rounded to fp32r for the PE) ----
            with tc.tile_pool(name="whhp", bufs=1) as whhp:
                whh_r = []
                for k in range(KT):
                    st = stage.tile([128, N_H], F32, tag="wstage")
                    nc.sync.dma_start(out=st[:, :], in_=Whh[k * 128:(k + 1) * 128, :])
                    wr = whhp.tile([128, N_H], F32R, tag=f"whh{k}")
                    nc.vector.tensor_copy(wr[:, :], st[:, :])
                    whh_r.append(wr)

                # ---- P2: recurrence ----
                # Per step: 4 psum banks of 512 cols; per bank 16 K-tile
                # matmuls, then (DVE add u) -> (ACT tanh) -> (DVE 32x32
                # transposes straight into the f32r stationary buffer).
                # The LAST bank's tail is split in two 256-col halves so the
                # next step's k=12..15 matmuls unblock sooner; everything
                # else pipelines under the next step's PE work.
                # State tiles ping-pong so the Hh store DMA and the u
                # prefetch DMAs (4 buffers, 4 steps ahead) are never on the
                # critical path.
                hT_a = persist.tile([128, KT * 32], F32R, tag="hTa")
                hT_b = persist.tile([128, KT * 32], F32R, tag="hTb")
                hT_s = persist.tile([128, KT * 32], F32, tag="hTs")
                th0 = persist.tile([32, N_H], F32, tag="th0")
                nc.vector.memset(th0[0:32, :], 0.0)
                if fast:
                    th1 = persist.tile([32, N_H], F32, tag="th1")
                    nc.vector.memset(th1[0:32, :], 0.0)
                    ths = [th0, th1]
                else:
                    ths = [th0, th0]
                u0 = persist.tile([BC, N_H], F32, tag="u0")
                u1 = persist.tile([BC, N_H], F32, tag="u1")
                if fast:
                    u2 = persist.tile([BC, N_H], F32, tag="u2")
                    u3 = persist.tile([BC, N_H], F32, tag="u3")
                    ubufs = [u0, u1, u2, u3]
                else:
                    ubufs = [u0, u1]
                NU = len(ubufs)
                if not fast:
                    alpha_sb = persist.tile([32, N_H], F32, tag="alpha")
                    hc0 = persist.tile([32, N_H], F32, tag="hc0")
                    hc1 = persist.tile([32, N_H], F32, tag="hc1")
                    nc.vector.memset(hc0[0:32, :], 0.0)
                    nc.vector.memset(hc1[0:32, :], 0.0)
                    nc.sync.dma_start(out=alpha_sb[:, :], in_=alpha_b[:, :])
                st = stage.tile([128, KT * 32], F32, tag="h0stage")
                nc.sync.dma_start(out=st[:, :], in_=hT0[:, :])

                with tc.tile_pool(name="psB", bufs=2, space="PSUM") as psB:

                    def step(tix, src, dst, u_tile, pf_tix, s_new, s_old,
                             th_new):
                        state = th_new if fast else s_new
                        sv = state[0:32, :].rearrange("p (i f c) -> p i f c",
                                                      f=4, c=32)

                        def tail(ps, ch, i0, ni):
                            # i0/ni are k-tile (128-col) block index/count
                            sl = slice(i0 * 128, (i0 + ni) * 128)
                            rsl = slice((i0 - 4 * ch) * 128,
                                        (i0 - 4 * ch + ni) * 128)
                            nc.vector.tensor_add(ps[:, rsl], ps[:, rsl],
                                                 u_tile[0:BC, sl])
                            nc.scalar.activation(th_new[0:BC, sl],
                                                 ps[:, rsl], AF.Tanh)
                            if not fast:
                                nc.vector.tensor_sub(th_new[0:BC, sl],
                                                     th_new[0:BC, sl],
                                                     s_old[0:BC, sl])
                                nc.vector.tensor_mul(th_new[0:BC, sl],
                                                     th_new[0:BC, sl],
                                                     alpha_sb[0:BC, sl])
                                nc.vector.tensor_add(s_new[0:BC, sl],
                                                     s_old[0:BC, sl],
                                                     th_new[0:BC, sl])
                            for g in range(4):
                                dv = hT_s[32 * g:32 * (g + 1), :].rearrange(
                                    "p (i c) -> p i c", c=32)
                                nc.vector.transpose(dv[:, i0:i0 + ni, :],
                                                    sv[:, i0:i0 + ni, g, :])
                            nc.vector.tensor_copy(
                                dst[:, 32 * i0:32 * (i0 + ni)],
                                hT_s[:, 32 * i0:32 * (i0 + ni)])

                        for ch in range(NCH):
                            sl = slice(ch * 512, (ch + 1) * 512)
                            ps = psB.tile([BC, 512], F32, tag=f"ps{ch}",
                                          name=f"ps{ch}")
                            for k in range(KT):
                                nc.tensor.matmul(
                                    ps[:, :], src[:, 32 * k:32 * k + BC],
                                    whh_r[k][:, sl],
                                    start=(k == 0), stop=(k == KT - 1))
                            if ch < NCH - 1:
                                tail(ps, ch, 4 * ch, 4)
                            else:
                                tail(ps, ch, 4 * ch, 2)
                                tail(ps, ch, 4 * ch + 2, 2)
                        # store state, prefetch u (both off critical path)
                        if P2_DMA:
                            nc.sync.dma_start(out=Hh_v[:, ds(tix, 1), :],
                                              in_=state[0:BC, :])
                            if pf_tix is not None:
                                nc.sync.dma_start(out=u_tile[0:BC, :],
                                                  in_=Uc_v[:, ds(pf_tix, 1), :])

                    def estep(t, pf_tix):
                        src, dst = (hT_a, hT_b) if t % 2 == 0 else (hT_b, hT_a)
                        s_new, s_old = ((hc0, hc1) if t % 2 == 0 else
                                        (hc1, hc0)) if not fast else (None, None)
                        th_new = ths[t % 2]
                        step(t, src, dst, ubufs[t % NU], pf_tix, s_new, s_old,
                             th_new)

                    for _rep in range(rep2):
                        nc.vector.tensor_copy(hT_a[:, :], st[:, :])
                        if not fast:
                            nc.sync.dma_start(out=hc1[:, :], in_=h0_b[:, :])
                        for j in range(NU):
                            nc.sync.dma_start(out=ubufs[j][0:BC, :],
                                              in_=Uc_v[:, ds(j, 1), :])
                        with tc.For_i(0, (T - 16) // 4, 1,
                                      staggered_reset=STAGGERED) as iv:
                            t0 = iv * 4
                            for j in range(4):
                                estep(t0 + j, t0 + j + NU)
                        for t in range(T - 16, T):
                            estep(t, t + NU if t + NU < T else None)

            # ---- P3: output projection + softmax ----
            with tc.tile_pool(name="p3", bufs=3) as p3, \
                 tc.tile_pool(name="psC", bufs=2, space="PSUM") as psC:
                why_r = []
                for k in range(KT):
                    st = stage.tile([128, N_OUT], F32, tag="whystage")
                    nc.sync.dma_start(out=st[:, :],
                                      in_=Why[k * 128:(k + 1) * 128, :])
                    wr = p3.tile([128, N_OUT], F32R, tag=f"why{k}", bufs=1)
                    nc.vector.tensor_copy(wr[:, :], st[:, :])
                    why_r.append(wr)
                ident = p3.tile([128, 128], F32, tag="ident", bufs=1)
                make_identity(nc, ident[:, :])
                if not fast:
                    by_sb = p3.tile([128, N_OUT], F32, tag="by", bufs=1)
                    nc.sync.dma_start(out=by_sb[:, :], in_=by_b[:, :])

                for i in list(range(BT // 128)) * rep3:
                    b = i // 4
                    t0 = (i % 4) * 128
                    htile = p3.tile([128, N_H], F32, tag="h3")
                    nc.sync.dma_start(out=htile[:, :],
                                      in_=Hh[t0:t0 + 128, b:b + 1, :])
                    hT3 = p3.tile([128, N_H], F32R, tag="hT3", bufs=2)
                    psy = psC.tile([128, N_OUT], F32, tag="psy")
                    for k in range(KT):
                        pst = psC.tile([128, 128], F32, tag=f"pst{k % 2}")
                        nc.tensor.transpose(
                            pst[:, :], htile[:, k * 128:(k + 1) * 128],
                            ident[:, :])
                        nc.vector.tensor_copy(
                            hT3[:, k * 128:(k + 1) * 128], pst[:, :])
                    for k in range(KT):
                        nc.tensor.matmul(
                            psy[:, :], hT3[:, k * 128:(k + 1) * 128],
                            why_r[k][:, :], start=(k == 0), stop=(k == KT - 1))
                    logits = p3.tile([128, N_OUT], F32, tag="logits")
                    if fast:
                        nc.vector.tensor_copy(logits[:, :], psy[:, :])
                    else:
                        nc.vector.tensor_add(logits[:, :], psy[:, :],
                                             by_sb[:, :])
                    nm = p3.tile([128, 1], F32, tag="nm")
                    nc.vector.tensor_reduce(nm[:, :], logits[:, :],
                                            axis=mybir.AxisListType.X,
                                            op=mybir.AluOpType.max)
                    nc.vector.tensor_scalar_mul(nm[:, :], nm[:, :], -1.0)
                    e = p3.tile([128, N_OUT], F32, tag="e")
                    s = p3.tile([128, 1], F32, tag="s")
                    nc.scalar.activation(e[:, :], logits[:, :], AF.Exp,
                                         bias=nm[:, :], scale=1.0,
                                         accum_out=s[:, :])
                    r = p3.tile([128, 1], F32, tag="r")
                    nc.vector.reciprocal(r[:, :], s[:, :])
                    yt = p3.tile([128, N_OUT], F32, tag="yt")
                    nc.vector.tensor_scalar_mul(yt[:, :], e[:, :], r[:, :])
                    nc.sync.dma_start(out=y[i * 128:(i + 1) * 128, :],
                                      in_=yt[:, :])

    nc.compile()
    return nc


_NC_CACHE = {}


def _get_nc(fast: bool):
    if fast not in _NC_CACHE:
        _NC_CACHE[fast] = _build(fast)
    return _NC_CACHE[fast]


def make_in_maps(inputs):
    """Build per-core input maps for the `fast` variant (setup_inputs data)."""
    u = np.ascontiguousarray(np.asarray(inputs["u"], dtype=np.float32))
    common, _ = _prep_common(
        inputs["W_uh"], inputs["W_hh"], inputs["W_hy"], inputs["b_h"],
        inputs["b_y"], inputs["h0"], inputs["tau"])
    in_maps = []
    for c in range(NCORES):
        uc = u[c * BC:(c + 1) * BC]
        uTc = np.ascontiguousarray(uc.reshape(BC * T, N_IN).T)
        in_maps.append({"uT": uTc, **common})
    return in_maps


def _prep_common(W_uh, W_hh, W_hy, b_h, b_y, h0, tau):
    W_uh = np.ascontiguousarray(np.asarray(W_uh, dtype=np.float32))
    W_hh = np.ascontiguousarray(np.asarray(W_hh, dtype=np.float32))
    W_hy = np.ascontiguousarray(np.asarray(W_hy, dtype=np.float32))
    b_h = np.asarray(b_h, dtype=np.float32)
    b_y = np.asarray(b_y, dtype=np.float32)
    h0 = np.asarray(h0, dtype=np.float32)
    tau = np.asarray(tau, dtype=np.float32)

    alpha = 1.0 / tau
    fast = bool(np.all(alpha == 1.0) and np.all(b_h == 0.0) and np.all(b_y == 0.0))

    h0c = np.clip(h0, -1.0, 1.0)
    hT0 = np.zeros((128, KT * 32), np.float32)
    for k in range(KT):
        hT0[:, 32 * k:32 * k + BC] = h0c[128 * k:128 * (k + 1)][:, None]

    common = {"Whh": W_hh, "Wuh": W_uh, "Why": W_hy, "hT0": hT0}
    if not fast:
        alpha_b = np.zeros((32, N_H), np.float32)
        alpha_b[:BC] = alpha[None, :]
        beta_b = np.zeros((32, N_H), np.float32)
        beta_b[:BC] = (1.0 - alpha)[None, :]
        h0_b = np.zeros((32, N_H), np.float32)
        h0_b[:BC] = h0c[None, :]
        common.update(
            alpha_b=alpha_b, beta_b=beta_b, h0_b=h0_b,
            bh_b=np.ascontiguousarray(np.broadcast_to(b_h[None, :], (128, N_H))),
            by_b=np.ascontiguousarray(np.broadcast_to(b_y[None, :], (128, N_OUT))),
        )
    return common, fast


def kernel(u, W_uh, W_hh, W_hy, b_h, b_y, h0, tau):
    u = np.ascontiguousarray(np.asarray(u, dtype=np.float32))
    common, fast = _prep_common(W_uh, W_hh, W_hy, b_h, b_y, h0, tau)
    nc = _get_nc(fast)

    in_maps = []
    for c in range(NCORES):
        uc = u[c * BC:(c + 1) * BC]                      # [BC, T, N_IN]
        uTc = np.ascontiguousarray(uc.reshape(BC * T, N_IN).T)
        in_maps.append({"uT": uTc, **common})

    res = run_bass_kernel_spmd(nc, in_maps, core_ids=list(range(NCORES)))
    ys = [res.results[c]["y"].reshape(BC, T, N_OUT) for c in range(NCORES)]
    return np.concatenate(ys, axis=0)



# revision 19
# speedup vs baseline: 1.7975x; 1.2882x over previous
"""Trainium2 Bass kernel for a leaky CTRNN (nn_RNN_25451976196554).

Math (per reference):
    alpha = 1/tau; h0c = clip(h0, -1, 1); h broadcast over batch
    per step t: pre = h @ W_hh + u_t @ W_uh + b_h
                h'  = (1-alpha)*h + alpha*tanh(pre)
                y_t = softmax(h' @ W_hy + b_y)

Strategy: data-parallel over batch (8 batch rows per core, 8 cores).
Tensor-parallelism (per the sharding hint) was measured and rejected: in
this environment a single 8-core AllGather costs ~270us and a critical-
path DMA round ~30us, so any per-step cross-core exchange dwarfs the
1.7us/step it would save on the PE.  Per core, three phases:
  P1: U_proj = u @ W_uh + b_h for all t (batched matmul, fp32r), -> DRAM.
  P2: sequential recurrence over T=512 steps. Batch-stationary matmuls:
      lhsT = h^T slices [128, 8] (stationary), rhs = W_hh K-tiles streamed
      as the moving operand in fp32r (1 cycle/row at N=512; measured
      ~15.3us/step pure-PE floor, cost ~ N_moving only). Per 512-wide
      psum bank: 16 matmuls, then DVE add-u (in place on PSUM), ACT tanh,
      DVE 32x32 transposes into an f32 staging buffer, and a rounding
      copy into the f32r stationary for the next step.  The last bank's
      tail is split in two 256-col halves so the next step's k=12..15
      matmuls unblock early; state/psum tiles ping-pong and u is
      prefetched 4 steps ahead so no DMA sits on the critical path.
      (fp8 DoubleRow would halve PE streaming time but fails the 2e-2
      gate: simulated rel err 2.6e-2. k-major matmul emission that
      interleaves PSUM accumulation groups is 2.4x slower - banks want
      16 consecutive matmuls.)
  P3: y = softmax(h_hist @ W_hy + b_y), batched over all (b, t).

Layouts (per core):
  uT   [256, 4096]  : u^T, columns indexed bt = b*T + t (b-major)
  hT   [128, 512]   : h^T packed; chunk k (h dims 128k..128k+128) lives in
                      cols [32k, 32k+8) (cols 32k+8..32k+32 are zero padding
                      written by the 32x32 block transposes).
  Uc   [T, 8, 2048] : U projection, per-step slice contiguous.
  Hh   [T, 8, 2048] : h state history, per-step slice contiguous.
  y    [4096, 256]  : output rows bt = b*T + t (b-major).
"""

import numpy as np

import concourse.bass as bass
import concourse.mybir as mybir
import concourse.tile as tile
from concourse import bacc
from concourse.bass import ds
from concourse.bass_utils import run_bass_kernel_spmd
from concourse.masks import make_identity

N_IN, N_H, N_OUT = 256, 2048, 256
BATCH, T = 64, 512
NCORES = 8
BC = BATCH // NCORES          # 8 batch rows per core
BT = BC * T                   # 4096
KT = N_H // 128               # 16 K tiles over the hidden dim
KIN = N_IN // 128             # 2 K tiles over the input dim
NCH = 4                       # 512-wide psum chunks over N_H
F32 = mybir.dt.float32
F32R = mybir.dt.float32r
AF = mybir.ActivationFunctionType
STAGGERED = True
P2_DMA = True  # timing experiments only; False skips h-store/u-prefetch
P3_BATCH = True  # batch 4 PE transposes per psum bank in P3


def _build(fast: bool, repeat: int = 1, ncores: int = NCORES,
           rep_phases: tuple = (1, 1, 1)):
    """repeat: re-run all phases `repeat` times inside the NEFF.
    rep_phases: additional per-phase multipliers (P1, P2, P3) for
    phase-cost attribution via the wall-clock slope method.
    Output is unaffected (each rep restarts from h0)."""
    rep1, rep2, rep3 = (repeat * r for r in rep_phases)
    nc = bacc.Bacc("TRN2", target_bir_lowering=False, debug=False,
                   num_devices=ncores)

    uT = nc.declare_dram_parameter("uT", [N_IN, BT], F32, isOutput=False)
    Whh = nc.declare_dram_parameter("Whh", [N_H, N_H], F32, isOutput=False)
    Wuh = nc.declare_dram_parameter("Wuh", [N_IN, N_H], F32, isOutput=False)
    Why = nc.declare_dram_parameter("Why", [N_H, N_OUT], F32, isOutput=False)
    hT0 = nc.declare_dram_parameter("hT0", [128, KT * 32], F32, isOutput=False)
    if not fast:
        alpha_b = nc.declare_dram_parameter("alpha_b", [32, N_H], F32, isOutput=False)
        beta_b = nc.declare_dram_parameter("beta_b", [32, N_H], F32, isOutput=False)
        bh_b = nc.declare_dram_parameter("bh_b", [128, N_H], F32, isOutput=False)
        by_b = nc.declare_dram_parameter("by_b", [128, N_OUT], F32, isOutput=False)
        h0_b = nc.declare_dram_parameter("h0_b", [32, N_H], F32, isOutput=False)
    y = nc.declare_dram_parameter("y", [BT, N_OUT], F32, isOutput=True)

    Uc = nc.dram_tensor("Uc", [T, BC, N_H], F32)
    Hh = nc.dram_tensor("Hh", [T, BC, N_H], F32)
    Uc_v = Uc.ap().rearrange("t b n -> b t n")
    Hh_v = Hh.ap().rearrange("t b n -> b t n")

    with tile.TileContext(nc) as tc:
        with tc.tile_pool(name="persist", bufs=1) as persist, \
             tc.tile_pool(name="stage", bufs=2) as stage:

            # ---- P1: U projection ----
            with tc.tile_pool(name="p1", bufs=3) as p1, \
                 tc.tile_pool(name="psA", bufs=2, space="PSUM") as psA:
                wuh_r = []
                for k in range(KIN):
                    st = stage.tile([128, N_H], F32, tag="wstage")
                    nc.sync.dma_start(out=st[:, :], in_=Wuh[k * 128:(k + 1) * 128, :])
                    wr = p1.tile([128, N_H], F32R, tag=f"wuh{k}", bufs=1)
                    nc.vector.tensor_copy(wr[:, :], st[:, :])
                    wuh_r.append(wr)
                if not fast:
                    bh_sb = p1.tile([128, N_H], F32, tag="bh", bufs=1)
                    nc.sync.dma_start(out=bh_sb[:, :], in_=bh_b[:, :])

                for i in list(range(BT // 128)) * rep1:
                    b = i // 4
                    t0 = (i % 4) * 128
                    uts = []
                    for k in range(KIN):
                        st = p1.tile([128, 128], F32, tag=f"ustage{k}")
                        nc.sync.dma_start(
                            out=st[:, :],
                            in_=uT[k * 128:(k + 1) * 128, i * 128:(i + 1) * 128])
                        ur = p1.tile([128, 128], F32R, tag=f"ur{k}")
                        nc.vector.tensor_copy(ur[:, :], st[:, :])
                        uts.append(ur)
                    acc = p1.tile([128, N_H], F32, tag="uacc")
                    for ch in range(NCH):
                        ps = psA.tile([128, 512], F32, tag=f"ps{ch}")
                        for k in range(KIN):
                            nc.tensor.matmul(
                                ps[:, :], uts[k][:, :],
                                wuh_r[k][:, ch * 512:(ch + 1) * 512],
                                start=(k == 0), stop=(k == KIN - 1))
                        if fast:
                            nc.vector.tensor_copy(
                                acc[:, ch * 512:(ch + 1) * 512], ps[:, :])
                        else:
                            nc.vector.tensor_add(
                                acc[:, ch * 512:(ch + 1) * 512], ps[:, :],
                                bh_sb[:, ch * 512:(ch + 1) * 512])
                    nc.sync.dma_start(
                        out=Uc[t0:t0 + 128, b:b + 1, :], in_=acc[:, :])

            # ---- resident W_hh (rounded to fp32r for the PE) ----
            with tc.tile_pool(name="whhp", bufs=1) as whhp:
                whh_r = []
                for k in range(KT):
                    st = stage.tile([128, N_H], F32, tag="wstage")
                    nc.sync.dma_start(out=st[:, :], in_=Whh[k * 128:(k + 1) * 128, :])
                    wr = whhp.tile([128, N_H], F32R, tag=f"whh{k}")
                    nc.vector.tensor_copy(wr[:, :], st[:, :])
                    whh_r.append(wr)

                # ---- P2: recurrence ----
                # Per step: 4 psum banks of 512 cols; per bank 16 K-tile
                # matmuls, then (DVE add u) -> (ACT tanh) -> (DVE 32x32
                # transposes straight into the f32r stationary buffer).
                # The LAST bank's tail is split in two 256-col halves so the
                # next step's k=12..15 matmuls unblock sooner; everything
                # else pipelines under the next step's PE work.
                # State tiles ping-pong so the Hh store DMA and the u
                # prefetch DMAs (4 buffers, 4 steps ahead) are never on the
                # critical path.
                hT_a = persist.tile([128, KT * 32], F32R, tag="hTa")
                hT_b = persist.tile([128, KT * 32], F32R, tag="hTb")
                hT_s = persist.tile([128, KT * 32], F32, tag="hTs")
                th0 = persist.tile([32, N_H], F32, tag="th0")
                nc.vector.memset(th0[0:32, :], 0.0)
                if fast:
                    th1 = persist.tile([32, N_H], F32, tag="th1")
                    nc.vector.memset(th1[0:32, :], 0.0)
                    ths = [th0, th1]
                else:
                    ths = [th0, th0]
                u0 = persist.tile([BC, N_H], F32, tag="u0")
                u1 = persist.tile([BC, N_H], F32, tag="u1")
                if fast:
                    u2 = persist.tile([BC, N_H], F32, tag="u2")
                    u3 = persist.tile([BC, N_H], F32, tag="u3")
                    ubufs = [u0, u1, u2, u3]
                else:
                    ubufs = [u0, u1]
                NU = len(ubufs)
                if not fast:
                    alpha_sb = persist.tile([32, N_H], F32, tag="alpha")
                    hc0 = persist.tile([32, N_H], F32, tag="hc0")
                    hc1 = persist.tile([32, N_H], F32, tag="hc1")
                    nc.vector.memset(hc0[0:32, :], 0.0)
                    nc.vector.memset(hc1[0:32, :], 0.0)
                    nc.sync.dma_start(out=alpha_sb[:, :], in_=alpha_b[:, :])
                st = stage.tile([128, KT * 32], F32, tag="h0stage")
                nc.sync.dma_start(out=st[:, :], in_=hT0[:, :])

                with tc.tile_pool(name="psB", bufs=2, space="PSUM") as psB:

                    def step(tix, src, dst, u_tile, pf_tix, s_new, s_old,
                             th_new):
                        state = th_new if fast else s_new
                        sv = state[0:32, :].rearrange("p (i f c) -> p i f c",
                                                      f=4, c=32)

                        def tail(ps, ch, i0, ni):
                            # i0/ni are k-tile (128-col) block index/count
                            sl = slice(i0 * 128, (i0 + ni) * 128)
                            rsl = slice((i0 - 4 * ch) * 128,
                                        (i0 - 4 * ch + ni) * 128)
                            nc.vector.tensor_add(ps[:, rsl], ps[:, rsl],
                                                 u_tile[0:BC, sl])
                            nc.scalar.activation(th_new[0:BC, sl],
                                                 ps[:, rsl], AF.Tanh)
                            if not fast:
                                nc.vector.tensor_sub(th_new[0:BC, sl],
                                                     th_new[0:BC, sl],
                                                     s_old[0:BC, sl])
                                nc.vector.tensor_mul(th_new[0:BC, sl],
                                                     th_new[0:BC, sl],
                                                     alpha_sb[0:BC, sl])
                                nc.vector.tensor_add(s_new[0:BC, sl],
                                                     s_old[0:BC, sl],
                                                     th_new[0:BC, sl])
                            for g in range(4):
                                dv = hT_s[32 * g:32 * (g + 1), :].rearrange(
                                    "p (i c) -> p i c", c=32)
                                nc.vector.transpose(dv[:, i0:i0 + ni, :],
                                                    sv[:, i0:i0 + ni, g, :])
                            nc.vector.tensor_copy(
                                dst[:, 32 * i0:32 * (i0 + ni)],
                                hT_s[:, 32 * i0:32 * (i0 + ni)])

                        for ch in range(NCH):
                            sl = slice(ch * 512, (ch + 1) * 512)
                            ps = psB.tile([BC, 512], F32, tag=f"ps{ch}",
                                          name=f"ps{ch}")
                            for k in range(KT):
                                nc.tensor.matmul(
                                    ps[:, :], src[:, 32 * k:32 * k + BC],
                                    whh_r[k][:, sl],
                                    start=(k == 0), stop=(k == KT - 1))
                            if ch < NCH - 1:
                                tail(ps, ch, 4 * ch, 4)
                            elif not fast:
                                tail(ps, ch, 4 * ch, 2)
                                tail(ps, ch, 4 * ch + 2, 2)
                            else:
                                # last bank: ONE wide add+tanh, then split
                                # transposes+copies. Splitting add/tanh too
                                # stalls the in-order DVE queue behind ACT
                                # (measured 18.3 -> 15.6 us/step in exp_pe).
                                sl = slice(ch * 512, (ch + 1) * 512)
                                nc.vector.tensor_add(ps[:, :], ps[:, :],
                                                     u_tile[0:BC, sl])
                                nc.scalar.activation(th_new[0:BC, sl],
                                                     ps[:, :], AF.Tanh)
                                for i0 in (4 * ch, 4 * ch + 2):
                                    for g in range(4):
                                        dv = hT_s[32 * g:32 * (g + 1),
                                                  :].rearrange(
                                            "p (i c) -> p i c", c=32)
                                        nc.vector.transpose(
                                            dv[:, i0:i0 + 2, :],
                                            sv[:, i0:i0 + 2, g, :])
                                    nc.vector.tensor_copy(
                                        dst[:, 32 * i0:32 * (i0 + 2)],
                                        hT_s[:, 32 * i0:32 * (i0 + 2)])
                        # store state, prefetch u (both off critical path)
                        if P2_DMA:
                            nc.sync.dma_start(out=Hh_v[:, ds(tix, 1), :],
                                              in_=state[0:BC, :])
                            if pf_tix is not None:
                                nc.sync.dma_start(out=u_tile[0:BC, :],
                                                  in_=Uc_v[:, ds(pf_tix, 1), :])

                    def estep(t, pf_tix):
                        src, dst = (hT_a, hT_b) if t % 2 == 0 else (hT_b, hT_a)
                        s_new, s_old = ((hc0, hc1) if t % 2 == 0 else
                                        (hc1, hc0)) if not fast else (None, None)
                        th_new = ths[t % 2]
                        step(t, src, dst, ubufs[t % NU], pf_tix, s_new, s_old,
                             th_new)

                    for _rep in range(rep2):
                        nc.vector.tensor_copy(hT_a[:, :], st[:, :])
                        if not fast:
                            nc.sync.dma_start(out=hc1[:, :], in_=h0_b[:, :])
                        for j in range(NU):
                            nc.sync.dma_start(out=ubufs[j][0:BC, :],
                                              in_=Uc_v[:, ds(j, 1), :])
                        with tc.For_i(0, (T - 16) // 4, 1,
                                      staggered_reset=STAGGERED) as iv:
                            t0 = iv * 4
                            for j in range(4):
                                estep(t0 + j, t0 + j + NU)
                        for t in range(T - 16, T):
                            estep(t, t + NU if t + NU < T else None)

            # ---- P3: output projection + softmax ----
            with tc.tile_pool(name="p3", bufs=3) as p3, \
                 tc.tile_pool(name="psC", bufs=2, space="PSUM") as psC:
                why_r = []
                for k in range(KT):
                    st = stage.tile([128, N_OUT], F32, tag="whystage")
                    nc.sync.dma_start(out=st[:, :],
                                      in_=Why[k * 128:(k + 1) * 128, :])
                    wr = p3.tile([128, N_OUT], F32R, tag=f"why{k}", bufs=1)
                    nc.vector.tensor_copy(wr[:, :], st[:, :])
                    why_r.append(wr)
                ident = p3.tile([128, 128], F32, tag="ident", bufs=1)
                make_identity(nc, ident[:, :])
                if not fast:
                    by_sb = p3.tile([128, N_OUT], F32, tag="by", bufs=1)
                    nc.sync.dma_start(out=by_sb[:, :], in_=by_b[:, :])

                for i in list(range(BT // 128)) * rep3:
                    b = i // 4
                    t0 = (i % 4) * 128
                    htile = p3.tile([128, N_H], F32, tag="h3")
                    nc.sync.dma_start(out=htile[:, :],
                                      in_=Hh[t0:t0 + 128, b:b + 1, :])
                    hT3 = p3.tile([128, N_H], F32R, tag="hT3", bufs=2)
                    psy = psC.tile([128, N_OUT], F32, tag="psy")
                    # 4 transposes into ONE full psum bank before switching:
                    # consecutive matmuls alternating banks pay a large
                    # pipeline-drain penalty (measured 2.4x in exp_pe).
                    if P3_BATCH:
                        for q in range(KT // 4):
                            pst = psC.tile([128, 512], F32, tag="pst")
                            for j in range(4):
                                k = 4 * q + j
                                nc.tensor.transpose(
                                    pst[:, j * 128:(j + 1) * 128],
                                    htile[:, k * 128:(k + 1) * 128],
                                    ident[:, :])
                            nc.vector.tensor_copy(
                                hT3[:, q * 512:(q + 1) * 512], pst[:, :])
                    else:
                        for k in range(KT):
                            pst = psC.tile([128, 128], F32,
                                           tag=f"pst{k % 2}", name="pst2")
                            nc.tensor.transpose(
                                pst[:, :], htile[:, k * 128:(k + 1) * 128],
                                ident[:, :])
                            nc.vector.tensor_copy(
                                hT3[:, k * 128:(k + 1) * 128], pst[:, :])
                    for k in range(KT):
                        nc.tensor.matmul(
                            psy[:, :], hT3[:, k * 128:(k + 1) * 128],
                            why_r[k][:, :], start=(k == 0), stop=(k == KT - 1))
                    logits = p3.tile([128, N_OUT], F32, tag="logits")
                    if fast:
                        nc.vector.tensor_copy(logits[:, :], psy[:, :])
                    else:
                        nc.vector.tensor_add(logits[:, :], psy[:, :],
                                             by_sb[:, :])
                    nm = p3.tile([128, 1], F32, tag="nm")
                    nc.vector.tensor_reduce(nm[:, :], logits[:, :],
                                            axis=mybir.AxisListType.X,
                                            op=mybir.AluOpType.max)
                    nc.vector.tensor_scalar_mul(nm[:, :], nm[:, :], -1.0)
                    e = p3.tile([128, N_OUT], F32, tag="e")
                    s = p3.tile([128, 1], F32, tag="s")
                    nc.scalar.activation(e[:, :], logits[:, :], AF.Exp,
                                         bias=nm[:, :], scale=1.0,
                                         accum_out=s[:, :])
                    r = p3.tile([128, 1], F32, tag="r")
                    nc.vector.reciprocal(r[:, :], s[:, :])
                    yt = p3.tile([128, N_OUT], F32, tag="yt")
                    nc.vector.tensor_scalar_mul(yt[:, :], e[:, :], r[:, :])
                    nc.sync.dma_start(out=y[i * 128:(i + 1) * 128, :],
                                      in_=yt[:, :])

    nc.compile()
    return nc


_NC_CACHE = {}


def _get_nc(fast: bool):
    if fast not in _NC_CACHE:
        _NC_CACHE[fast] = _build(fast)
    return _NC_CACHE[fast]


def make_in_maps(inputs):
    """Build per-core input maps for the `fast` variant (setup_inputs data)."""
    u = np.ascontiguousarray(np.asarray(inputs["u"], dtype=np.float32))
    common, _ = _prep_common(
        inputs["W_uh"], inputs["W_hh"], inputs["W_hy"], inputs["b_h"],
        inputs["b_y"], inputs["h0"], inputs["tau"])
    in_maps = []
    for c in range(NCORES):
        uc = u[c * BC:(c + 1) * BC]
        uTc = np.ascontiguousarray(uc.reshape(BC * T, N_IN).T)
        in_maps.append({"uT": uTc, **common})
    return in_maps


def _prep_common(W_uh, W_hh, W_hy, b_h, b_y, h0, tau):
    W_uh = np.ascontiguousarray(np.asarray(W_uh, dtype=np.float32))
    W_hh = np.ascontiguousarray(np.asarray(W_hh, dtype=np.float32))
    W_hy = np.ascontiguousarray(np.asarray(W_hy, dtype=np.float32))
    b_h = np.asarray(b_h, dtype=np.float32)
    b_y = np.asarray(b_y, dtype=np.float32)
    h0 = np.asarray(h0, dtype=np.float32)
    tau = np.asarray(tau, dtype=np.float32)

    alpha = 1.0 / tau
    fast = bool(np.all(alpha == 1.0) and np.all(b_h == 0.0) and np.all(b_y == 0.0))

    h0c = np.clip(h0, -1.0, 1.0)
    hT0 = np.zeros((128, KT * 32), np.float32)
    for k in range(KT):
        hT0[:, 32 * k:32 * k + BC] = h0c[128 * k:128 * (k + 1)][:, None]

    common = {"Whh": W_hh, "Wuh": W_uh, "Why": W_hy, "hT0": hT0}
    if not fast:
        alpha_b = np.zeros((32, N_H), np.float32)
        alpha_b[:BC] = alpha[None, :]
        beta_b = np.zeros((32, N_H), np.float32)
        beta_b[:BC] = (1.0 - alpha)[None, :]
        h0_b = np.zeros((32, N_H), np.float32)
        h0_b[:BC] = h0c[None, :]
        common.update(
            alpha_b=alpha_b, beta_b=beta_b, h0_b=h0_b,
            bh_b=np.ascontiguousarray(np.broadcast_to(b_h[None, :], (128, N_H))),
            by_b=np.ascontiguousarray(np.broadcast_to(b_y[None, :], (128, N_OUT))),
        )
    return common, fast


def kernel(u, W_uh, W_hh, W_hy, b_h, b_y, h0, tau):
    u = np.ascontiguousarray(np.asarray(u, dtype=np.float32))
    common, fast = _prep_common(W_uh, W_hh, W_hy, b_h, b_y, h0, tau)
    nc = _get_nc(fast)

    in_maps = []
    for c in range(NCORES):
        uc = u[c * BC:(c + 1) * BC]                      # [BC, T, N_IN]
        uTc = np.ascontiguousarray(uc.reshape(BC * T, N_IN).T)
        in_maps.append({"uT": uTc, **common})

    res = run_bass_kernel_spmd(nc, in_maps, core_ids=list(range(NCORES)))
    ys = [res.results[c]["y"].reshape(BC, T, N_OUT) for c in range(NCORES)]
    return np.concatenate(ys, axis=0)

